# revision 1
# baseline (speedup 1.0000x reference)
"""DeepSeek-MLA prefill kernel for 8 Trainium2 NeuronCores (head-parallel).

Contract: kernel(**inputs) takes the FULL unsharded inputs from
setup_inputs() and returns the FULL [1, 2048, 4096] float32 output.

Sharding (hardcoded for B=1, S=2048, H=4096, NH=32):
  - LoRA down-projections: sequence-parallel (256 rows/core), then an
    on-chip AllGather of the (transposed, rms-normed, roped) activations.
  - Up-projections + attention: head-parallel, 4 heads/core.
  - Output projection: AllGather of per-head attention outputs, then each
    core computes a 512-column slice of out @ wo.T; host concatenates.
All matmuls run in bf16 with fp32 PSUM accumulation. Softmax runs on
transposed scores (keys on partitions): ACT exp straight from PSUM into
bf16 probs, denominators via a ones-vector matmul, normalization deferred
to the [128, S] per-head output via GpSimd partition-broadcast. Causal
masking skips fully-masked key-chunks and trims masked query columns from
every stage including the score matmuls themselves.
"""

import numpy as np
import ml_dtypes

import concourse.bass as bass
import concourse.mybir as mybir
import concourse.tile as tile
from concourse import bacc
from concourse.bass_utils import run_bass_kernel_spmd
from concourse.masks import make_identity

BF16 = mybir.dt.bfloat16
F32 = mybir.dt.float32
AF = mybir.ActivationFunctionType

NCORE = 8
B, S, H = 1, 2048, 4096
NH = 32
DN, DR, DV = 128, 64, 128  # qk_nope, qk_rope, v dims
LQ, LKV = 1536, 512
EPS = 1e-6
HL = NH // NCORE          # heads per core = 4
SC = S // NCORE           # local seq rows = 256
ACT_D = LQ + LKV + DR     # 2112 gathered activation feature dim
NEG = -1e5

_compiled = {}


def _build(collectives=True):
    nc = bacc.Bacc("TRN2", target_bir_lowering=False, debug=False,
                   num_devices=NCORE)

    x_t = nc.declare_dram_parameter("x_t", [H, SC], BF16, isOutput=False)
    wq_at = nc.declare_dram_parameter("wq_at", [H, LQ], BF16, isOutput=False)
    wkv_at = nc.declare_dram_parameter("wkv_at", [H, LKV + DR], BF16, isOutput=False)
    wqbn_t = nc.declare_dram_parameter("wqbn_t", [LQ, HL * DN], BF16, isOutput=False)
    wqbr_t = nc.declare_dram_parameter("wqbr_t", [LQ, HL * DR], BF16, isOutput=False)
    wkvbn_t = nc.declare_dram_parameter("wkvbn_t", [LKV, HL * DN], BF16, isOutput=False)
    wkvbv_t = nc.declare_dram_parameter("wkvbv_t", [LKV, HL * DV], BF16, isOutput=False)
    wo_t = nc.declare_dram_parameter("wo_t", [NH * DV, 512], BF16, isOutput=False)
    cos_k = nc.declare_dram_parameter("cos_k", [SC, DR // 2], F32, isOutput=False)
    sin_k = nc.declare_dram_parameter("sin_k", [SC, DR // 2], F32, isOutput=False)
    cos_r = nc.declare_dram_parameter("cos_r", [S, HL * DR // 2], F32, isOutput=False)
    sin_r = nc.declare_dram_parameter("sin_r", [S, HL * DR // 2], F32, isOutput=False)
    dmask = nc.declare_dram_parameter("dmask", [128, 128], F32, isOutput=False)
    out_c = nc.declare_dram_parameter("out_c", [S, 512], F32, isOutput=True)

    KQ = LQ // 128    # 12 lora k-chunks (q)
    KKV = LKV // 128  # 4
    KH = H // 128     # 32
    NT = S // 128     # 16 seq tiles
    NG = S // 512     # 4 seq groups

    with tile.TileContext(nc) as tc:
        with tc.tile_pool(name="consts", bufs=1) as consts, \
             tc.tile_pool(name="dram", bufs=1, space="DRAM") as dram:
            ident = consts.tile([128, 128], BF16)
            make_identity(nc, ident)
            ones_k = consts.tile([128, 1], BF16)
            nc.vector.memset(ones_k, 1.0)
            ones_m = consts.tile([1, 128], F32)
            nc.vector.memset(ones_m, 1.0)
            dmask_sb = consts.tile([128, 128], F32)
            nc.sync.dma_start(out=dmask_sb, in_=dmask[:, :])
            eps_sb = consts.tile([128, 1], F32)
            nc.vector.memset(eps_sb, EPS)

            bounce_act = dram.tile([ACT_D, SC], BF16)
            gath_act = dram.tile([NCORE, ACT_D, SC], BF16,
                                 addr_space="Shared" if collectives else "Local")
            bounce_out = dram.tile([HL * DV, S], BF16)
            gath_out = dram.tile([NCORE, HL * DV, S], BF16,
                                 addr_space="Shared" if collectives else "Local")

            # ---------------- Phase 1: LoRA down-proj on local rows ------
            with tc.tile_pool(name="p1w", bufs=4) as p1w, \
                 tc.tile_pool(name="p1ps", bufs=1, space="PSUM") as p1ps, \
                 tc.tile_pool(name="p1tps", bufs=2, space="PSUM") as p1tps, \
                 tc.tile_pool(name="p1sb", bufs=3) as p1sb, \
                 tc.tile_pool(name="p1acc", bufs=2) as p1acc:
                x_sb = p1sb.tile([128, KH, SC], BF16, tag="xsb", bufs=1)
                nc.sync.dma_start(out=x_sb, in_=x_t.rearrange("(kk p) s -> p kk s", p=128))
                cosk_sb = p1sb.tile([128, 2, DR // 2], F32, tag="cosk", bufs=1)
                sink_sb = p1sb.tile([128, 2, DR // 2], F32, tag="sink", bufs=1)
                nc.sync.dma_start(out=cosk_sb, in_=cos_k.rearrange("(mt p) i -> p mt i", p=128))
                nc.sync.dma_start(out=sink_sb, in_=sin_k.rearrange("(mt p) i -> p mt i", p=128))

                # sweep 1: q lora (stream wq_at once, both mt tiles)
                psq = [[p1ps.tile([128, 512], F32, tag=f"pp{mt * 3 + g}",
                                  name=f"psq{mt}{g}")
                        for g in range(3)] for mt in range(2)]
                for kk in range(KH):
                    w_sb = p1w.tile([128, LQ], BF16, tag="w")
                    eng = nc.sync if kk % 2 == 0 else nc.scalar
                    eng.dma_start(out=w_sb,
                                  in_=wq_at[kk * 128:(kk + 1) * 128, :])
                    st, sp = kk == 0, kk == KH - 1
                    for mt in range(2):
                        lhs = x_sb[:, kk, mt * 128:(mt + 1) * 128]
                        for g in range(3):
                            nc.tensor.matmul(psq[mt][g], lhs,
                                             w_sb[:, g * 512:(g + 1) * 512],
                                             start=st, stop=sp)

                act_local = []
                for mt in range(2):
                    actl = p1acc.tile([128, ACT_D], BF16, tag=f"actl{mt}",
                                      name=f"actl{mt}")
                    act_local.append(actl)
                    ps_list = psq[mt]
                    # rms norm q (over 1536)
                    sq = p1sb.tile([128, 512], F32, tag="sq")
                    r3 = p1sb.tile([128, 3], F32, tag="r3")
                    for g in range(3):
                        nc.scalar.activation(sq, ps_list[g], AF.Square,
                                             accum_out=r3[:, g:g + 1])
                    ssum = p1sb.tile([128, 1], F32, tag="ssum")
                    nc.vector.tensor_reduce(ssum, r3, mybir.AxisListType.X,
                                            mybir.AluOpType.add)
                    rtq = p1sb.tile([128, 1], F32, tag="rtq")
                    nc.scalar.activation(rtq, ssum, AF.Sqrt, bias=eps_sb,
                                         scale=1.0 / LQ)
                    rstdq = p1sb.tile([128, 1], F32, tag="rstdq")
                    nc.vector.reciprocal(rstdq, rtq)
                    for g in range(3):
                        nc.vector.tensor_scalar_mul(
                            actl[:, g * 512:(g + 1) * 512], ps_list[g], rstdq)

                # sweep 2: kv lora (stream wkv_at once)
                pskv = [p1ps.tile([128, 512], F32, tag=f"pp{mt}",
                                  name=f"pskv{mt}") for mt in range(2)]
                pskpe = [p1ps.tile([128, DR], F32, tag=f"pp{2 + mt}",
                                   name=f"pskpe{mt}") for mt in range(2)]
                for kk in range(KH):
                    wv_sb = p1w.tile([128, LKV + DR], BF16, tag="wv")
                    eng = nc.sync if kk % 2 == 0 else nc.scalar
                    eng.dma_start(out=wv_sb,
                                  in_=wkv_at[kk * 128:(kk + 1) * 128, :])
                    st, sp = kk == 0, kk == KH - 1
                    for mt in range(2):
                        lhs = x_sb[:, kk, mt * 128:(mt + 1) * 128]
                        nc.tensor.matmul(pskv[mt], lhs, wv_sb[:, :512],
                                         start=st, stop=sp)
                        nc.tensor.matmul(pskpe[mt], lhs, wv_sb[:, 512:],
                                         start=st, stop=sp)

                for mt in range(2):
                    actl = act_local[mt]
                    # rms norm kv (over 512)
                    sqv = p1sb.tile([128, 512], F32, tag="sqv")
                    ssv = p1sb.tile([128, 1], F32, tag="ssv")
                    nc.scalar.activation(sqv, pskv[mt], AF.Square,
                                         accum_out=ssv)
                    rtv = p1sb.tile([128, 1], F32, tag="rtv")
                    nc.scalar.activation(rtv, ssv, AF.Sqrt, bias=eps_sb,
                                         scale=1.0 / LKV)
                    rstdv = p1sb.tile([128, 1], F32, tag="rstdv")
                    nc.vector.reciprocal(rstdv, rtv)
                    nc.vector.tensor_scalar_mul(actl[:, LQ:LQ + 512],
                                                pskv[mt], rstdv)
                    # rope k_pe (no norm)
                    kv3 = pskpe[mt].rearrange("p (i two) -> p i two", two=2)
                    x1, x2 = kv3[:, :, 0], kv3[:, :, 1]
                    cs, sn = cosk_sb[:, mt, :], sink_sb[:, mt, :]
                    t1 = p1sb.tile([128, DR // 2], F32, tag="t1")
                    t2 = p1sb.tile([128, DR // 2], F32, tag="t2")
                    ko = actl[:, LQ + 512:].rearrange("p (i two) -> p i two",
                                                      two=2)
                    nc.vector.tensor_mul(t1, x1, cs)
                    nc.vector.tensor_mul(t2, x2, sn)
                    nc.vector.tensor_sub(ko[:, :, 0], t1, t2)
                    nc.vector.tensor_mul(t1, x1, sn)
                    nc.vector.tensor_mul(t2, x2, cs)
                    nc.vector.tensor_add(ko[:, :, 1], t1, t2)

                # transpose local activations -> [ACT_D, SC] and DMA to bounce
                NFT = (ACT_D + 127) // 128  # 17
                for mt in range(SC // 128):
                    for ft in range(NFT):
                        fs = ft * 128
                        fw = min(128, ACT_D - fs)
                        tp = p1tps.tile([128, 128], BF16, tag="tp")
                        nc.tensor.transpose(tp[:fw, :], act_local[mt][:, fs:fs + fw],
                                            ident)
                        cp = p1sb.tile([128, 128], BF16, tag="cp")
                        nc.scalar.copy(cp[:fw, :], tp[:fw, :])
                        nc.sync.dma_start(
                            out=bounce_act[fs:fs + fw, mt * 128:(mt + 1) * 128],
                            in_=cp[:fw, :])

            if collectives:
                nc.gpsimd.collective_compute(
                    "AllGather", mybir.AluOpType.bypass,
                    replica_groups=[list(range(NCORE))],
                    ins=[bounce_act.opt()], outs=[gath_act.opt()])
            else:
                for r in range(NCORE):
                    nc.gpsimd.dma_start(out=gath_act[r], in_=bounce_act[:, :])

            # ---------------- Phase 2: up-projections (4 local heads) ----
            from contextlib import ExitStack
            with tc.tile_pool(name="acts", bufs=1) as acts, \
                 tc.tile_pool(name="attn", bufs=1) as attn_pool:
              with ExitStack() as p2stack:
                p2w = p2stack.enter_context(tc.tile_pool(name="p2w", bufs=1))
                p2ps = p2stack.enter_context(tc.tile_pool(name="p2ps", bufs=3, space="PSUM"))
                p2tps = p2stack.enter_context(tc.tile_pool(name="p2tps", bufs=2, space="PSUM"))
                p2sb = p2stack.enter_context(tc.tile_pool(name="p2sb", bufs=3))
                actT = []
                for ft in range(NFT):
                    fw = min(128, ACT_D - ft * 128)
                    a = acts.tile([128, NCORE, SC], BF16, tag=f"actT{ft}")
                    nc.sync.dma_start(
                        out=a[:fw],
                        in_=gath_act[:, ft * 128:ft * 128 + fw, :]
                            .rearrange("r p s -> p r s"))
                    actT.append(a.rearrange("p r s -> p (r s)"))

                wqbn_sb = p2w.tile([128, KQ, HL * DN], BF16, tag="wqbn")
                nc.sync.dma_start(out=wqbn_sb,
                                  in_=wqbn_t.rearrange("(kk p) n -> p kk n", p=128))
                wqbr_sb = p2w.tile([128, KQ, HL * DR], BF16, tag="wqbr")
                nc.sync.dma_start(out=wqbr_sb,
                                  in_=wqbr_t.rearrange("(kk p) n -> p kk n", p=128))
                wkvbn_sb = p2w.tile([128, KKV, HL * DN], BF16, tag="wkvbn")
                nc.sync.dma_start(out=wkvbn_sb,
                                  in_=wkvbn_t.rearrange("(kk p) n -> p kk n", p=128))
                wkvbv_sb = p2w.tile([128, KKV, HL * DV], BF16, tag="wkvbv")
                nc.sync.dma_start(out=wkvbv_sb,
                                  in_=wkvbv_t.rearrange("(kk p) n -> p kk n", p=128))
                cosr_sb = p2w.tile([128, NT, HL * DR // 2], F32, tag="cosr")
                nc.sync.dma_start(out=cosr_sb,
                                  in_=cos_r.rearrange("(mt p) i -> p mt i", p=128))
                sinr_sb = p2w.tile([128, NT, HL * DR // 2], F32, tag="sinr")
                nc.sync.dma_start(out=sinr_sb,
                                  in_=sin_r.rearrange("(mt p) i -> p mt i", p=128))

                # q_nope.T  [128, S] per head
                qnT = [attn_pool.tile([128, S], BF16, tag=f"qnT{h}", name=f"qnT{h}") for h in range(HL)]
                for h in range(HL):
                    for g in range(NG):
                        ps = p2ps.tile([128, 512], F32, tag="ps2")
                        for kk in range(KQ):
                            nc.tensor.matmul(ps, wqbn_sb[:, kk, h * 128:(h + 1) * 128],
                                             actT[kk][:, g * 512:(g + 1) * 512],
                                             start=kk == 0, stop=kk == KQ - 1)
                        nc.scalar.copy(qnT[h][:, g * 512:(g + 1) * 512], ps)
                # k_nope.T  [128, S] per head
                knT = [attn_pool.tile([128, S], BF16, tag=f"knT{h}", name=f"knT{h}") for h in range(HL)]
                for h in range(HL):
                    for g in range(NG):
                        ps = p2ps.tile([128, 512], F32, tag="ps2")
                        for kk in range(KKV):
                            nc.tensor.matmul(ps, wkvbn_sb[:, kk, h * 128:(h + 1) * 128],
                                             actT[KQ + kk][:, g * 512:(g + 1) * 512],
                                             start=kk == 0, stop=kk == KKV - 1)
                        nc.scalar.copy(knT[h][:, g * 512:(g + 1) * 512], ps)
                # v natural [S, HL*DV] as 16 tiles [128, 512]
                v_sb = [attn_pool.tile([128, HL * DV], BF16, tag=f"v{mt}", name=f"v{mt}")
                        for mt in range(NT)]
                for mt in range(NT):
                    ps = p2ps.tile([128, 512], F32, tag="ps2")
                    for kk in range(KKV):
                        nc.tensor.matmul(ps, actT[KQ + kk][:, mt * 128:(mt + 1) * 128],
                                         wkvbv_sb[:, kk, :],
                                         start=kk == 0, stop=kk == KKV - 1)
                    nc.scalar.copy(v_sb[mt], ps)
                # q_pe natural, rope, then transpose into [128(2h), S] tiles
                qpeT = [attn_pool.tile([64, S], BF16, tag=f"qpeT{i}", name=f"qpeT{i}") for i in range(HL)]
                for mt in range(NT):
                    ps = p2ps.tile([128, HL * DR], F32, tag="psqpe")
                    for kk in range(KQ):
                        nc.tensor.matmul(ps, actT[kk][:, mt * 128:(mt + 1) * 128],
                                         wqbr_sb[:, kk, :],
                                         start=kk == 0, stop=kk == KQ - 1)
                    pv = ps.rearrange("p (h i two) -> p h i two", h=HL, two=2)
                    x1, x2 = pv[:, :, :, 0], pv[:, :, :, 1]
                    cs = cosr_sb[:, mt, :].rearrange("p (h i) -> p h i", h=HL)
                    sn = sinr_sb[:, mt, :].rearrange("p (h i) -> p h i", h=HL)
                    qp = p2sb.tile([128, HL * DR], BF16, tag="qp")
                    qpv = qp.rearrange("p (h i two) -> p h i two", h=HL, two=2)
                    t1 = p2sb.tile([128, HL * DR // 2], F32, tag="t1")
                    t1v = t1.rearrange("p (h i) -> p h i", h=HL)
                    t2 = p2sb.tile([128, HL * DR // 2], F32, tag="t2")
                    t2v = t2.rearrange("p (h i) -> p h i", h=HL)
                    nc.vector.tensor_mul(t1v, x1, cs)
                    nc.vector.tensor_mul(t2v, x2, sn)
                    nc.vector.tensor_sub(qpv[:, :, :, 0], t1v, t2v)
                    nc.vector.tensor_mul(t1v, x1, sn)
                    nc.vector.tensor_mul(t2v, x2, cs)
                    nc.vector.tensor_add(qpv[:, :, :, 1], t1v, t2v)
                    for h in range(HL):
                        tp = p2tps.tile([64, 128], BF16, tag="tpq")
                        nc.tensor.transpose(tp, qp[:, h * DR:(h + 1) * DR], ident)
                        nc.scalar.copy(
                            qpeT[h][:, mt * 128:(mt + 1) * 128], tp)

              kpeT = actT[NFT - 1]  # [64 used, S]

              # ------------- Phase 3: causal attention, 4 heads --------
              with tc.tile_pool(name="p3ps_s", bufs=4, space="PSUM") as p3s, \
                   tc.tile_pool(name="p3ps_o", bufs=2, space="PSUM") as p3o, \
                   tc.tile_pool(name="p3ps_m", bufs=1, space="PSUM") as p3m, \
                   tc.tile_pool(name="p3sb", bufs=3) as p3sb, \
                   tc.tile_pool(name="outT", bufs=1) as outp:
                  outT = [outp.tile([128, S], BF16, tag=f"outT{h}", name=f"outT{h}")
                          for h in range(HL)]
                  for h in range(HL):
                      qn, kn = qnT[h], knT[h]
                      qp = qpeT[h]
                      for g in range(NG):
                          ps_o = p3o.tile([128, 512], F32, tag="ps_o")
                          ps_sum = p3m.tile([1, 512], F32, tag="ps_sum", bufs=2)
                          nk = 4 * g + 4
                          for c in range(nk):
                              s = c - 4 * g
                              off = 128 * s if s >= 0 else 0
                              ps_s = p3s.tile([128, 512], F32, tag="ps_s")
                              # columns < off are fully causal-masked: skip
                              # them in the score matmuls (exp/v-mm already do)
                              nc.tensor.matmul(
                                  ps_s[:, off:],
                                  kn[:, c * 128:(c + 1) * 128],
                                  qn[:, g * 512 + off:(g + 1) * 512],
                                  start=True, stop=False)
                              nc.tensor.matmul(
                                  ps_s[:, off:],
                                  kpeT[:64, c * 128:(c + 1) * 128],
                                  qp[:, g * 512 + off:(g + 1) * 512],
                                  start=False, stop=True)
                              if s >= 0:
                                  nc.vector.tensor_add(
                                      ps_s[:, off:off + 128],
                                      ps_s[:, off:off + 128], dmask_sb)
                              pt = p3sb.tile([128, 512], BF16, tag="pt", bufs=4)
                              nc.scalar.activation(pt[:, off:], ps_s[:, off:],
                                                   AF.Exp)
                              nc.tensor.matmul(
                                  ps_o[:, off:],
                                  v_sb[c][:, h * 128:(h + 1) * 128],
                                  pt[:, off:],
                                  start=c == 0, stop=c == nk - 1)
                              nc.tensor.matmul(
                                  ps_sum[:, off:], ones_k, pt[:, off:],
                                  start=c == 0, stop=c == nk - 1)
                          sums = p3sb.tile([1, 512], F32, tag="sums")
                          nc.vector.tensor_copy(sums, ps_sum)
                          rec = p3sb.tile([1, 512], F32, tag="rec")
                          nc.vector.reciprocal(rec, sums)
                          rb = p3sb.tile([128, 512], F32, tag="rb")
                          nc.gpsimd.partition_broadcast(rb, rec)
                          nc.vector.tensor_mul(
                              outT[h][:, g * 512:(g + 1) * 512], ps_o, rb)
                      nc.sync.dma_start(
                          out=bounce_out[h * 128:(h + 1) * 128, :], in_=outT[h])

            if collectives:
                nc.gpsimd.collective_compute(
                    "AllGather", mybir.AluOpType.bypass,
                    replica_groups=[list(range(NCORE))],
                    ins=[bounce_out.opt()], outs=[gath_out.opt()])
            else:
                for r in range(NCORE):
                    nc.gpsimd.dma_start(out=gath_out[r], in_=bounce_out[:, :])

            # ---------------- Phase 4: output projection slice ----------
            with tc.tile_pool(name="p4w", bufs=1) as p4w, \
                 tc.tile_pool(name="p4a", bufs=3) as p4a, \
                 tc.tile_pool(name="p4ps", bufs=2, space="PSUM") as p4ps, \
                 tc.tile_pool(name="p4sb", bufs=3) as p4sb:
                wo_sb = p4w.tile([128, KH, 512], BF16, tag="wo")
                nc.sync.dma_start(out=wo_sb,
                                  in_=wo_t.rearrange("(kk p) n -> p kk n", p=128))
                for mt in range(NT):
                    ps = p4ps.tile([128, 512], F32, tag="psf")
                    a_sb = p4a.tile([128, KH, 128], BF16, tag="a", bufs=3)
                    nc.sync.dma_start(
                        out=a_sb,
                        in_=gath_out[:, :, mt * 128:(mt + 1) * 128]
                            .rearrange("r (q4 p) s -> p r q4 s", p=128)
                            .rearrange("p r q4 s -> p (r q4) s"))
                    for kk in range(KH):
                        nc.tensor.matmul(ps, a_sb[:, kk, :], wo_sb[:, kk, :],
                                         start=kk == 0, stop=kk == KH - 1)
                    f_sb = p4sb.tile([128, 512], F32, tag="f")
                    nc.vector.tensor_copy(f_sb, ps)
                    nc.sync.dma_start(out=out_c[mt * 128:(mt + 1) * 128, :],
                                      in_=f_sb)

    nc.compile()
    return nc


def _prep(x, wq_a, q_norm_w, wq_b, wkv_a, kv_norm_w, wkv_b, wo):
    bf = ml_dtypes.bfloat16
    f32 = np.float32
    x2 = np.asarray(x, f32).reshape(S, H)
    xT = np.ascontiguousarray(x2.T).astype(bf)                    # [H, S]
    wq_aT = np.ascontiguousarray(np.asarray(wq_a, f32).T).astype(bf)
    wkv_aT = np.ascontiguousarray(np.asarray(wkv_a, f32).T).astype(bf)

    scale = 1.0 / np.sqrt(np.float32(DN + DR))
    wq_b_eff = (np.asarray(wq_b, f32) * np.asarray(q_norm_w, f32)[None, :]
                * scale).reshape(NH, DN + DR, LQ)
    wkv_b_eff = (np.asarray(wkv_b, f32)
                 * np.asarray(kv_norm_w, f32)[None, :]).reshape(NH, DN + DV, LKV)

    # rope tables (mirror reference fp32 math)
    freqs = 1.0 / (10000.0 ** (np.arange(0, DR, 2, dtype=f32) / DR))
    t = np.arange(S, dtype=f32)
    ang = np.outer(t, freqs)                                      # [S, 32]
    cos, sin = np.cos(ang).astype(f32), np.sin(ang).astype(f32)
    cos_rep = np.tile(cos, (1, HL)).astype(f32)                   # [S, 128]
    sin_rep = np.tile(sin, (1, HL)).astype(f32)

    dm = np.where(np.arange(128)[:, None] > np.arange(128)[None, :],
                  np.float32(NEG), np.float32(0.0))

    in_maps = []
    for c in range(NCORE):
        hs = slice(c * HL, (c + 1) * HL)
        wqbn = wq_b_eff[hs, :DN, :].reshape(HL * DN, LQ)
        wqbr = wq_b_eff[hs, DN:, :].reshape(HL * DR, LQ)
        wkvbn = wkv_b_eff[hs, :DN, :].reshape(HL * DN, LKV)
        wkvbv = wkv_b_eff[hs, DN:, :].reshape(HL * DV, LKV)
        in_maps.append({
            "x_t": np.ascontiguousarray(xT[:, c * SC:(c + 1) * SC]),
            "wq_at": wq_aT,
            "wkv_at": wkv_aT,
            "wqbn_t": np.ascontiguousarray(wqbn.T).astype(bf),
            "wqbr_t": np.ascontiguousarray(wqbr.T).astype(bf),
            "wkvbn_t": np.ascontiguousarray(wkvbn.T).astype(bf),
            "wkvbv_t": np.ascontiguousarray(wkvbv.T).astype(bf),
            "wo_t": np.ascontiguousarray(
                np.asarray(wo, f32)[c * 512:(c + 1) * 512, :].T).astype(bf),
            "cos_k": np.ascontiguousarray(cos[c * SC:(c + 1) * SC]),
            "sin_k": np.ascontiguousarray(sin[c * SC:(c + 1) * SC]),
            "cos_r": cos_rep,
            "sin_r": sin_rep,
            "dmask": dm,
        })
    return in_maps


def kernel(x, wq_a, q_norm_w, wq_b, wkv_a, kv_norm_w, wkv_b, wo,
           _trace=False):
    if "nc" not in _compiled:
        _compiled["nc"] = _build()
    nc = _compiled["nc"]
    in_maps = _prep(x, wq_a, q_norm_w, wq_b, wkv_a, kv_norm_w, wkv_b, wo)
    try:
        res = run_bass_kernel_spmd(nc, in_maps, list(range(NCORE)),
                                   trace=_trace)
    except Exception:
        # transient NRT/device wedge: one retry after a short pause
        import time as _time
        _time.sleep(15)
        res = run_bass_kernel_spmd(nc, in_maps, list(range(NCORE)),
                                   trace=_trace)
    _compiled["last_result"] = res
    out = np.concatenate([res.results[c]["out_c"] for c in range(NCORE)],
                         axis=1)
    return out.reshape(B, S, NH * DV).astype(np.float32)

